# revision 1
# baseline (speedup 1.0000x reference)
"""DenseGAT layer kernel for 8 Trainium2 NeuronCores (Bass/Tile).

Reference computation (per batch b):
    h = x @ W.T                               [N, D], heads H=4, DH=64
    src_i = h[i, hd]·a_src[h],  dst_j = h[j, hd]·a_dst[h]
    e_ij = leaky_relu(src_i + dst_j, 0.2); masked by adj; softmax over j
    out = attn @ h; out = out @ Wo.T + bo; LN(x + out)

Sharding: 8 cores = (batch b, query half). Core c handles b = c//2,
queries i0 = (c%2)*1024 .. +1024. Keys/values (all 2048 nodes of batch b)
are computed on-core from x[b].

Key algebraic identity used on-chip (C_i cancels in the softmax):
    exp(leaky(s_i + d_j)) = max(exp(s_i+d_j), exp(0.2(s_i+d_j)))
                          = B_j * C_i * max(P_i, Q_j)
    with P=exp(0.8 s), Q=exp(-0.8 d), B=exp(d), C=exp(0.2 s)
so the unnormalized masked weight is  w'_ji = adj_ji * B_j * max(P_i, Q_j),
one dual-op tensor_scalar (4x bf16) + one tensor_tensor mask (2x bf16) per
[128, 1024] tile. No N^2 transcendentals.

Layout: keys j on partitions, queries i on the free dim; the AV matmul
(lhsT = [h | 1], rhs = w') yields out^T in PSUM with the softmax sums in
row 64, and out^T is directly the lhsT of the output projection.
"""

import sys

sys.path.insert(0, "/opt/trn_rl_repo")

from contextlib import ExitStack

import ml_dtypes
import numpy as np

B, N, D, H = 4, 2048, 256, 4
DH = D // H
NQ = N // 2  # queries per core
NCORES = 8
LN_EPS = 1e-5
KT = D // 128  # 2 contraction tiles
NT = N // 128  # 16 node tiles
QT = NQ // 128  # 8 query tiles
F32 = np.float32
BF16 = ml_dtypes.bfloat16

_BUILT = {}  # cache: flags -> nc
MASK_GPS_JT = (5, 10)  # j-tiles whose mask TT runs on GPSIMD


def _build(skip_bo=False, skip_gamma=False, skip_beta=False):
    import concourse.bass as bass
    import concourse.mybir as mybir
    import concourse.tile as tile
    from concourse import bacc
    from concourse.masks import make_identity
    from concourse.tile_rust import add_dep_helper

    fp32 = mybir.dt.float32
    bf16 = mybir.dt.bfloat16
    fp16 = mybir.dt.float16
    Alu = mybir.AluOpType
    Act = mybir.ActivationFunctionType

    nc = bacc.Bacc(None, target_bir_lowering=False, debug=False)

    # ---- DRAM I/O (per-core data; same NEFF on all 8 cores) ----
    xT = nc.dram_tensor("xT", [D, N], fp32, kind="ExternalInput")  # x[b].T, keys rolled: queries first
    xs = nc.dram_tensor("xs", [NQ, D], fp16, kind="ExternalInput")  # residual rows
    adjT = nc.dram_tensor("adjT", [N, NQ], bf16, kind="ExternalInput")  # adj[i,:].T rolled
    wT = nc.dram_tensor("wT", [D, D], bf16, kind="ExternalInput")  # W.T
    vsd = nc.dram_tensor("vsd", [D, 8], fp32, kind="ExternalInput")  # [V_src | V_dst]
    woT = nc.dram_tensor("woT", [D, D], fp16, kind="ExternalInput")  # Wo.T
    bo = nc.dram_tensor("bo", [1, D], fp32, kind="ExternalInput")
    gamma = nc.dram_tensor("gamma", [1, D], fp32, kind="ExternalInput")
    beta = nc.dram_tensor("beta", [1, D], fp32, kind="ExternalInput")
    out = nc.dram_tensor("out", [NQ, D], fp32, kind="ExternalOutput")

    with tile.TileContext(nc) as tc, ExitStack() as ctx:
        singles = ctx.enter_context(tc.tile_pool(name="singles", bufs=1))
        work = ctx.enter_context(tc.tile_pool(name="work", bufs=5))
        wwork = ctx.enter_context(tc.tile_pool(name="wwork", bufs=5))
        small = ctx.enter_context(tc.tile_pool(name="small", bufs=4))
        psum_g = ctx.enter_context(tc.tile_pool(name="psum_g", bufs=4, space="PSUM"))
        psum_o = ctx.enter_context(tc.tile_pool(name="psum_o", bufs=2, space="PSUM"))
        dram = ctx.enter_context(tc.tile_pool(name="dram", bufs=1, space="DRAM"))

        def bcast_row(row_ap, parts=128):
            # stride-0 partition broadcast of a [1, F] DRAM row
            return bass.AP(
                tensor=row_ap.tensor,
                offset=row_ap.offset,
                ap=[[0, parts]] + [list(d) for d in row_ap.ap[1:]],
            )

        # ---- constants / weights ----
        wT_sb = singles.tile([128, KT, D], bf16, tag="wT")
        nc.sync.dma_start(out=wT_sb, in_=wT.rearrange("(k p) d -> p k d", p=128))
        vsd_sb = singles.tile([128, KT, 8], fp32, tag="vsd")
        nc.sync.dma_start(out=vsd_sb, in_=vsd.rearrange("(k p) d -> p k d", p=128))
        woT_sb = singles.tile([128, KT, D], fp16, tag="woT")
        nc.sync.dma_start(out=woT_sb, in_=woT.rearrange("(k p) d -> p k d", p=128))
        eps_sb = singles.tile([128, 1], fp32, tag="eps")
        nc.vector.memset(eps_sb, LN_EPS)
        warm = small.tile([1, 1], fp32, tag="warm")
        nc.vector.memset(warm, 0.0)
        nc.scalar.activation(out=warm, in_=warm, func=Act.Exp)
        ones64 = singles.tile([1, 64], fp32, tag="ones64")
        nc.vector.memset(ones64, 1.0)
        identf = singles.tile([128, 128], fp16, tag="identf")
        make_identity(nc, identf)
        identb = singles.tile([128, 128], bf16, tag="identb")
        make_identity(nc, identb)
        if not skip_bo:
            bo_bc = singles.tile([128, D], fp32, tag="bo")
            nc.sync.dma_start(out=bo_bc, in_=bcast_row(bo[:, :]))
        if not skip_gamma:
            gamma_bc = singles.tile([128, D], fp32, tag="gamma")
            nc.sync.dma_start(out=gamma_bc, in_=bcast_row(gamma[:, :]))
        if not skip_beta:
            beta_bc = singles.tile([128, D], fp32, tag="beta")
            nc.sync.dma_start(out=beta_bc, in_=bcast_row(beta[:, :]))

        # ---- x[b].T chunked loads (per-piece tiles) + bf16 converts ----
        NCH = 4
        CH = N // NCH
        dma_engs = [nc.sync, nc.gpsimd, nc.sync, nc.gpsimd]
        xTf = {}
        xTb = {}

        def load_xt(k, c, defer=False):
            xTf[k, c] = singles.tile([128, CH], fp32, tag=f"xf{k}{c}", name=f"xf{k}{c}")
            di = dma_engs[(2 * k + c) % 4].dma_start(
                out=xTf[k, c], in_=xT[bass.ts(k, 128), bass.ts(c, CH)]
            )
            if defer:
                add_dep_helper(di.ins, pb_gate.ins, sync=True, reason="defer-bulk")
            xTb[k, c] = singles.tile([128, CH], bf16, tag=f"xb{k}{c}", name=f"xb{k}{c}")
            if c >= 2:
                nc.gpsimd.tensor_copy(out=xTb[k, c], in_=xTf[k, c])
            else:
                nc.vector.tensor_copy(out=xTb[k, c], in_=xTf[k, c])

        def xf(k, t):  # [128, 128] n-tile slice views
            return xTf[k, t * 128 // CH][:, bass.ts(t % (CH // 128), 128)]

        def xb(k, t):
            return xTb[k, t * 128 // CH][:, bass.ts(t % (CH // 128), 128)]

        for c in range(2):  # query columns first: src projections can start
            for k in range(KT):
                load_xt(k, c)

        # ---- adjacency (per-tile) ----
        adj_sb = {}
        for t in range(NT):
            adj_sb[t] = singles.tile([128, NQ], bf16, tag=f"adj{t}", name=f"adj{t}")
        for t in range(4):
            dma_engs[t % 4].dma_start(out=adj_sb[t], in_=adjT[bass.ts(t, 128), :])

        # ---- node projections: h1 = [h | ones] bf16, B/Q per-partition cols ----
        h1 = {}
        Bcol = {}
        Qcol = {}
        acol = singles.tile([128, H, QT], bf16, tag="acol")

        def do_psd(t):
            psd = psum_g.tile([128, 8], fp32, tag="g")
            for k in range(KT):
                nc.tensor.matmul(
                    psd,
                    lhsT=xf(k, t),
                    rhs=vsd_sb[:, k, :],
                    start=(k == 0),
                    stop=(k == KT - 1),
                )
            if t < QT:  # A = exp(0.8 src) of this core's queries, for transposing
                nc.scalar.activation(
                    out=acol[:, :, t], in_=psd[:, 0:4], func=Act.Exp, scale=0.8
                )
            Bcol[t] = singles.tile([128, H], fp32, tag=f"B{t}", name=f"B{t}")
            Qcol[t] = singles.tile([128, H], fp32, tag=f"Q{t}", name=f"Q{t}")
            nc.scalar.activation(out=Bcol[t], in_=psd[:, 4:8], func=Act.Exp)
            nc.scalar.activation(out=Qcol[t], in_=psd[:, 4:8], func=Act.Exp, scale=-0.8)

        def do_ph(t):
            ph = psum_g.tile([128, D], fp32, tag="g")
            for k in range(KT):
                nc.tensor.matmul(
                    ph,
                    lhsT=xb(k, t),
                    rhs=wT_sb[:, k, :],
                    start=(k == 0),
                    stop=(k == KT - 1),
                )
            h1[t] = singles.tile([128, H, DH + 1], bf16, tag=f"h1{t}", name=f"h1{t}")
            nc.vector.memset(h1[t][:, :, DH : DH + 1], 1.0)
            nc.scalar.copy(
                out=h1[t][:, :, 0:DH], in_=ph.rearrange("p (h d) -> p h d", h=H)
            )

        for t in range(QT):
            do_psd(t)

        # transpose exp'd query src values -> rows (bf16 PE transpose)
        atr = psum_g.tile([H * QT, 128], bf16, tag="g")
        nc.tensor.transpose(
            out=atr, in_=acol.rearrange("p h t -> p (h t)"), identity=identb
        )
        arow = singles.tile([H * QT, 128], bf16, tag="arow")
        nc.vector.tensor_copy(out=arow, in_=atr)
        arow_dr = dram.tile([H * QT, 128], bf16, tag="adr")
        nc.sync.dma_start(out=arow_dr, in_=arow)
        Pb = {}
        for h in range(H):
            Pb[h] = singles.tile([128, NQ], bf16, tag=f"Pb{h}", name=f"Pb{h}")
            row = arow_dr[h * QT : (h + 1) * QT, :]
            pb_inst = nc.scalar.dma_start(
                out=Pb[h],
                in_=bass.AP(
                    tensor=row.tensor, offset=row.offset, ap=[[0, 128], [1, NQ]]
                ),
            )
            if h == 0:
                pb_gate = pb_inst

        # rest of the bulk loads (late adj tiles deferred behind the Pb path)
        def defer(inst):
            add_dep_helper(inst.ins, pb_gate.ins, sync=True, reason="defer-bulk")

        for k in range(KT):
            load_xt(k, 2)
        for t in range(4, 6):
            dma_engs[t % 4].dma_start(out=adj_sb[t], in_=adjT[bass.ts(t, 128), :])
        for k in range(KT):
            load_xt(k, 3)
        for t in range(6, NT):
            defer(
                dma_engs[t % 4].dma_start(out=adj_sb[t], in_=adjT[bass.ts(t, 128), :])
            )
        xs_sb = singles.tile([128, QT, D], fp16, tag="xs")
        defer(nc.sync.dma_start(out=xs_sb, in_=xs.rearrange("(t p) d -> p t d", p=128)))

        for t in range(QT):
            do_ph(t)
        for t in range(QT, NT):
            do_ph(t)
            do_psd(t)

        # ---- main attention loops ----
        outT = singles.tile([128, KT, NQ], fp16, tag="outT")
        pp = {}
        for h in range(H):
            po = psum_o.tile([128, NQ], fp32, tag="po")
            for t in range(NT):
                eng = nc.gpsimd if t in MASK_GPS_JT else nc.vector
                tt = work.tile([128, NQ], bf16, tag="tt")
                nc.vector.tensor_scalar(
                    out=tt,
                    in0=Pb[h],
                    scalar1=Qcol[t][:, h : h + 1],
                    scalar2=Bcol[t][:, h : h + 1],
                    op0=Alu.max,
                    op1=Alu.mult,
                )
                ww = wwork.tile([128, NQ], bf16, tag="ww")
                eng.tensor_tensor(out=ww, in0=tt, in1=adj_sb[t], op=Alu.mult)
                for ch in range(NQ // 512):
                    nc.tensor.matmul(
                        po[0 : DH + 1, bass.ts(ch, 512)],
                        lhsT=h1[t][:, h, :],
                        rhs=ww[:, bass.ts(ch, 512)],
                        start=(t == 0),
                        stop=(t == NT - 1),
                    )
            # normalize: rows 0..63 are out^T(head h), row 64 the softmax sums
            rrow = small.tile([1, NQ], fp32, tag="rrow")
            nc.vector.reciprocal(out=rrow, in_=po[DH : DH + 1, :])
            ou = work.tile([64, NQ], fp32, tag="ou")
            nc.scalar.copy(out=ou, in_=po[0:DH, :])  # frees po for the next head
            oslice = outT[(h % 2) * 64 : (h % 2) * 64 + 64, h // 2, :]
            if h < H - 1:
                rrow_dr = dram.tile([1, NQ], fp32, tag="rdr")
                nc.scalar.dma_start(out=rrow_dr, in_=rrow)
                rbc = work.tile([64, NQ], fp32, tag="rbc")
                nc.scalar.dma_start(out=rbc, in_=bcast_row(rrow_dr[:, :], parts=64))
                nc.gpsimd.tensor_tensor(out=oslice, in0=ou, in1=rbc, op=Alu.mult)
            else:
                # latency-critical last head: broadcast r via a K=1 PE matmul
                rps = psum_o.tile([64, NQ], fp32, tag="po")
                for ch in range(NQ // 512):
                    nc.tensor.matmul(
                        rps[:, bass.ts(ch, 512)],
                        lhsT=ones64,
                        rhs=rrow[:, bass.ts(ch, 512)],
                    )
                nc.vector.tensor_tensor(out=oslice, in0=ou, in1=rps, op=Alu.mult)
            if h == 1:
                # heads 0/1 complete -> start the k=0 projection matmuls
                for t in range(QT // 2):
                    pp[t] = psum_g.tile([128, D], fp32, tag="g", name=f"pp{t}")
                    nc.tensor.matmul(
                        pp[t],
                        lhsT=outT[:, 0, bass.ts(t, 128)],
                        rhs=woT_sb[:, 0, :],
                        start=True,
                        stop=False,
                    )
                # pre-switch the ACT table set used by layernorm's Sqrt
                nc.scalar.activation(out=warm, in_=warm, func=Act.Sqrt)

        # ---- output projection + residual + layernorm ----
        for t in range(QT):
            if t in pp:
                p = pp[t]
                nc.tensor.matmul(
                    p,
                    lhsT=outT[:, 1, bass.ts(t, 128)],
                    rhs=woT_sb[:, 1, :],
                    start=False,
                    stop=False,
                )
            else:
                p = psum_g.tile([128, D], fp32, tag="g", name=f"pp{t}")
                for k in range(KT):
                    nc.tensor.matmul(
                        p,
                        lhsT=outT[:, k, bass.ts(t, 128)],
                        rhs=woT_sb[:, k, :],
                        start=(k == 0),
                        stop=False,
                    )
            nc.tensor.matmul(p, lhsT=identf, rhs=xs_sb[:, t, :], start=False, stop=True)
            y = p
            if not skip_bo:
                y = work.tile([128, D], fp32, tag="y")
                nc.vector.tensor_tensor(out=y, in0=p, in1=bo_bc, op=Alu.add)
            stats = small.tile([128, 6], fp32, tag="stats")
            nc.vector.bn_stats(out=stats, in_=y)
            mv = small.tile([128, 2], fp32, tag="mv")
            nc.vector.bn_aggr(out=mv, in_=stats)
            sq = small.tile([128, 1], fp32, tag="sq")
            nc.scalar.activation(
                out=sq, in_=mv[:, 1:2], func=Act.Sqrt, bias=eps_sb, scale=1.0
            )
            rstd = small.tile([128, 1], fp32, tag="rstd")
            nc.vector.reciprocal(out=rstd, in_=sq)
            xh = work.tile([128, D], fp32, tag="xh")
            nc.vector.tensor_scalar(
                out=xh,
                in0=y,
                scalar1=mv[:, 0:1],
                scalar2=rstd,
                op0=Alu.subtract,
                op1=Alu.mult,
            )
            if not skip_gamma:
                nc.vector.tensor_tensor(out=xh, in0=xh, in1=gamma_bc, op=Alu.mult)
            if not skip_beta:
                nc.vector.tensor_tensor(out=xh, in0=xh, in1=beta_bc, op=Alu.add)
            nc.sync.dma_start(out=out[bass.ts(t, 128), :], in_=xh)

    nc.finalize()
    return nc


def _host_prep(inputs):
    x = np.asarray(inputs["x"], F32)
    adj = np.asarray(inputs["adj"])
    W = np.asarray(inputs["W"], F32)
    a_src = np.asarray(inputs["a_src"], F32)
    a_dst = np.asarray(inputs["a_dst"], F32)
    Wo = np.asarray(inputs["Wo"], F32)
    bo = np.asarray(inputs["bo"], F32).reshape(1, D)
    gamma = np.asarray(inputs["gamma"], F32).reshape(1, D)
    beta = np.asarray(inputs["beta"], F32).reshape(1, D)

    # per-head folded projection vectors: dst = x @ V_dst, src = x @ V_src
    V_dst = np.stack([a_dst[h] @ W[h * DH : (h + 1) * DH, :] for h in range(H)], 1)
    V_src = np.stack([a_src[h] @ W[h * DH : (h + 1) * DH, :] for h in range(H)], 1)
    vsd = np.concatenate([V_src, V_dst], axis=1).astype(F32)  # [D, 8]

    wT = np.ascontiguousarray(W.T).astype(BF16)
    woT = np.ascontiguousarray(Wo.T).astype(np.float16)

    in_maps = []
    for c in range(NCORES):
        b, half = divmod(c, 2)
        i0 = half * NQ
        # roll key order so this core's queries are keys 0..NQ-1
        perm = np.concatenate([np.arange(i0, i0 + NQ), np.arange(0, i0), np.arange(i0 + NQ, N)])
        xb = x[b]  # [N, D]
        xT_roll = np.ascontiguousarray(xb[perm].T).astype(F32)  # [D, N]
        adj_sl = adj[i0 : i0 + NQ, :]  # queries rows
        adjT_roll = np.ascontiguousarray(adj_sl[:, perm].T).astype(BF16)  # [N, NQ]
        in_maps.append(
            {
                "xT": xT_roll,
                "xs": np.ascontiguousarray(xb[i0 : i0 + NQ]).astype(np.float16),
                "adjT": adjT_roll,
                "wT": wT,
                "vsd": vsd,
                "woT": woT,
                "bo": bo,
                "gamma": gamma,
                "beta": beta,
            }
        )
    return in_maps


def kernel(**inputs) -> np.ndarray:
    from concourse.bass_utils import run_bass_kernel_spmd

    flags = (
        bool(np.all(np.asarray(inputs["bo"]) == 0.0)),
        bool(np.all(np.asarray(inputs["gamma"]) == 1.0)),
        bool(np.all(np.asarray(inputs["beta"]) == 0.0)),
    )
    if flags not in _BUILT:
        _BUILT[flags] = _build(*flags)
    nc = _BUILT[flags]

    in_maps = _host_prep(inputs)
    res = run_bass_kernel_spmd(nc, in_maps, core_ids=list(range(NCORES)))
    full = np.empty((B, N, D), F32)
    for c in range(NCORES):
        b, half = divmod(c, 2)
        full[b, half * NQ : (half + 1) * NQ] = res.results[c]["out"]
    return full

